# revision 12
# baseline (speedup 1.0000x reference)
"""Trainium2 Bass kernel for nn_Attention (B=4, N=2048, DIM=512, H=8).

Sharding: 8 cores = (batch b, seq-half s). Each core computes attention
outputs for queries [s*1024, (s+1)*1024) of batch b, all 8 heads, plus
the output projection for those rows. Outputs are disjoint -> host
gather is a pure concatenation (no reduction).

Per-core dataflow (all layouts chosen so no on-device transposes of
large tensors are ever needed):
  q_T [512,1024]  = (SCALE*wq) @ x_chunk.T    (features x queries)
  k_T [512,2048]  = wk @ x.T                  (features x keys)
  v   [2048, 512] = x @ wv.T                  (keys x features)
  per head h: scores_T[k,q] = k_h @ q_h.T     (keys on partitions)
       p = exp(scores_T) * exp(mask).T        (mask add via exp-multiply)
       U.T[d,q] += v_h.T @ p  (PSUM accum over key tiles, heads col-packed)
       sums[q] += ones.T @ p  (N=1 matmuls, queries on partitions)
  Uhat = U * (1/sums broadcast via transpose + rank-1 matmul)
  out[q,:] = Uhat.T @ proj_w.T + bias
"""
import functools
import numpy as np
import ml_dtypes
from contextlib import ExitStack

import concourse.bass as bass
import concourse.tile as tile
from concourse import bacc, mybir
from concourse.bass_utils import run_bass_kernel_spmd
from concourse.masks import make_identity

F32 = mybir.dt.float32
F32R = mybir.dt.float32r
BF16 = mybir.dt.bfloat16
AF = mybir.ActivationFunctionType

B, N, DIM, H, D = 4, 2048, 512, 8, 64
SCALE = D ** -0.5
NQ = N // 2          # queries per core
NKT = N // 128       # key tiles (16)
NCORES = 8


def build(dbg=False, dbg_pair=0):
    nc = bacc.Bacc("TRN2", target_bir_lowering=False, debug=False,
                   num_devices=NCORES)
    xT = nc.dram_tensor("xT", [DIM, N], F32R, kind="ExternalInput").ap()
    wqT = nc.dram_tensor("wqT", [DIM, DIM], F32R, kind="ExternalInput").ap()
    wkT = nc.dram_tensor("wkT", [DIM, DIM], F32R, kind="ExternalInput").ap()
    wvT = nc.dram_tensor("wvT", [DIM, DIM], F32R, kind="ExternalInput").ap()
    projT = nc.dram_tensor("projT", [DIM, DIM], F32R, kind="ExternalInput").ap()
    biasb = nc.dram_tensor("biasb", [128, DIM], F32, kind="ExternalInput").ap()
    expmT = nc.dram_tensor("expmT", [N, NQ], BF16, kind="ExternalInput").ap()
    indD = nc.dram_tensor("indD", [2, 128], F32R, kind="ExternalInput").ap()
    out = nc.dram_tensor("out", [NQ, DIM], F32, kind="ExternalOutput").ap()
    if dbg:
        d_q = nc.dram_tensor("d_q", [128, NQ], F32, kind="ExternalOutput").ap()
        d_k = nc.dram_tensor("d_k", [128, N], F32, kind="ExternalOutput").ap()
        d_v = nc.dram_tensor("d_v", [128, DIM], F32, kind="ExternalOutput").ap()
        d_ph = nc.dram_tensor("d_ph", [128, NQ], F32, kind="ExternalOutput").ap()
        d_sum = nc.dram_tensor("d_sum", [128, 16], F32, kind="ExternalOutput").ap()
        d_rr = nc.dram_tensor("d_rr", [2, 1024], F32, kind="ExternalOutput").ap()
        d_bc = nc.dram_tensor("d_bc", [128, 512], F32, kind="ExternalOutput").ap()
        d_u = nc.dram_tensor("d_u", [128, NQ], F32, kind="ExternalOutput").ap()
        d_uh = nc.dram_tensor("d_uh", [128, NQ], F32, kind="ExternalOutput").ap()
        d_uh1 = nc.dram_tensor("d_uh1", [128, NQ], F32, kind="ExternalOutput").ap()
        d_uh2 = nc.dram_tensor("d_uh2", [128, NQ], F32, kind="ExternalOutput").ap()
        d_uh3 = nc.dram_tensor("d_uh3", [128, NQ], F32, kind="ExternalOutput").ap()

    with tile.TileContext(nc) as tc, ExitStack() as ctx:
        # ---- pools ----
        wp = ctx.enter_context(tc.tile_pool(name="wp", bufs=1))
        kv = ctx.enter_context(tc.tile_pool(name="kv", bufs=1))
        small = ctx.enter_context(tc.tile_pool(name="small", bufs=2))
        small2 = ctx.enter_context(tc.tile_pool(name="small2", bufs=1))
        osb = ctx.enter_context(tc.tile_pool(name="osb", bufs=2))
        ps_stage = ctx.enter_context(tc.tile_pool(name="ps_stage", bufs=2, space="PSUM"))
        ps_out = ctx.enter_context(tc.tile_pool(name="ps_out", bufs=1, space="PSUM"))
        ps_misc = ctx.enter_context(tc.tile_pool(name="ps_misc", bufs=2, space="PSUM"))

        # ---- constants ----
        ident = wp.tile([128, 128], F32, name="ident", tag="ident")
        make_identity(nc, ident[:])
        ones_bf = wp.tile([128, 1], BF16, name="ones_bf", tag="ones_bf")
        nc.vector.memset(ones_bf[:], 1.0)
        zcol_bf = wp.tile([1, 128], BF16, name="zcol_bf", tag="zcol_bf")
        nc.vector.memset(zcol_bf[:], 0.0)
        zrow_bf = wp.tile([1, 512], BF16, name="zrow_bf", tag="zrow_bf")
        nc.vector.memset(zrow_bf[:], 1.0)
        ind = wp.tile([2, 128], F32R, name="ind", tag="ind")
        nc.sync.dma_start(ind[:], indD[:])

        # ---- weight / persistent loads ----
        pj_sb = [wp.tile([128, DIM], F32R, name=f"pj{kc}", tag=f"pj{kc}") for kc in range(4)]
        for kc in range(4):
            sl = slice(kc * 128, (kc + 1) * 128)
            nc.sync.dma_start(pj_sb[kc][:], projT[sl, :])
        bias_sb = wp.tile([128, DIM], F32, name="bias_sb", tag="bias_sb")
        nc.sync.dma_start(bias_sb[:], biasb[:])

        q_sb = [kv.tile([128, NQ], F32R, name=f"q{m}", tag=f"q{m}") for m in range(4)]
        k_sb = [kv.tile([128, N], F32R, name=f"k{m}", tag=f"k{m}") for m in range(4)]
        v_sb = [kv.tile([128, DIM], BF16, name=f"v{kt}", tag=f"v{kt}") for kt in range(NKT)]
        em_sb = [kv.tile([128, NQ], BF16, name=f"em{kt}", tag=f"em{kt}") for kt in range(NKT)]
        for kt in range(NKT):
            nc.sync.dma_start(em_sb[kt][:], expmT[kt * 128:(kt + 1) * 128, :])
        uhat = [kv.tile([128, NQ], F32R, name=f"uh{p}", tag=f"uh{p}") for p in range(4)]

        # ---- phase 1: projections ----
        with tc.tile_pool(name="xp", bufs=1) as xp:
            x_sb = [xp.tile([128, N], F32R, name=f"x{kc}", tag=f"x{kc}") for kc in range(4)]
            wq_sb = [xp.tile([128, DIM], F32R, name=f"wq{kc}", tag=f"wq{kc}") for kc in range(4)]
            wk_sb = [xp.tile([128, DIM], F32R, name=f"wk{kc}", tag=f"wk{kc}") for kc in range(4)]
            wv_sb = [xp.tile([128, DIM], F32R, name=f"wv{kc}", tag=f"wv{kc}") for kc in range(4)]
            for kc in range(4):
                sl = slice(kc * 128, (kc + 1) * 128)
                nc.sync.dma_start(x_sb[kc][:], xT[sl, :])
                nc.sync.dma_start(wq_sb[kc][:], wqT[sl, :])
                nc.sync.dma_start(wk_sb[kc][:], wkT[sl, :])
                nc.sync.dma_start(wv_sb[kc][:], wvT[sl, :])

            # q_T: [512, 1024] by feature tile m
            for m in range(4):
                ms = slice(m * 128, (m + 1) * 128)
                ps = ps_stage.tile([128, NQ], F32, name=f"psq{m}", tag="stage")
                for c in range(2):
                    cs = slice(c * 512, (c + 1) * 512)
                    for kc in range(4):
                        nc.tensor.matmul(ps[:, cs], wq_sb[kc][:, ms],
                                         x_sb[kc][:, cs],
                                         start=(kc == 0), stop=(kc == 3))
                nc.vector.tensor_copy(q_sb[m][:], ps[:])

            # k_T: [512, 2048]
            for m in range(4):
                ms = slice(m * 128, (m + 1) * 128)
                for half in range(2):
                    ps = ps_stage.tile([128, NQ], F32, name=f"psk{m}_{half}",
                                       tag="stage")
                    for c2 in range(2):
                        cs_o = slice(c2 * 512, (c2 + 1) * 512)
                        cs_x = slice(half * 1024 + c2 * 512,
                                     half * 1024 + (c2 + 1) * 512)
                        for kc in range(4):
                            nc.tensor.matmul(ps[:, cs_o], wk_sb[kc][:, ms],
                                             x_sb[kc][:, cs_x],
                                             start=(kc == 0), stop=(kc == 3))
                    nc.vector.tensor_copy(
                        k_sb[m][:, half * 1024:(half + 1) * 1024], ps[:])

            # v: [2048, 512] natural
            for kt in range(NKT):
                ks = slice(kt * 128, (kt + 1) * 128)
                ps = ps_misc.tile([128, DIM], F32, name=f"psv{kt}", tag="misc")
                for kc in range(4):
                    nc.tensor.matmul(ps[:], x_sb[kc][:, ks], wv_sb[kc][:],
                                     start=(kc == 0), stop=(kc == 3))
                nc.vector.tensor_copy(v_sb[kt][:], ps[:])

        if dbg:
            nc.sync.dma_start(d_q[:], q_sb[0][:].bitcast(F32))
            nc.sync.dma_start(d_k[:], k_sb[0][:].bitcast(F32))
            dvf = osb.tile([128, DIM], F32, name="dvf", tag="ob")
            nc.vector.tensor_copy(dvf[:], v_sb[0][:])
            nc.sync.dma_start(d_v[:], dvf[:])

        # ---- phase 2: attention per head-pair ----
        with tc.tile_pool(name="praw", bufs=3) as praw_p, \
             tc.tile_pool(name="phat", bufs=6) as phat_p:
            for pair in range(4):
                out_ps = ps_out.tile([128, NQ], F32, name=f"o{pair}", tag="out")
                sums_ps = ps_misc.tile([128, 512], F32, name=f"s{pair}",
                                       tag="misc")
                # open ONE accumulation group over the sums bank (zeros it)
                nc.tensor.matmul(sums_ps[:], zcol_bf[:], zrow_bf[:],
                                 start=True, stop=False)
                for kt in range(NKT):
                    kts = slice(kt * 128, (kt + 1) * 128)
                    for hi in range(2):
                        h = 2 * pair + hi
                        po = hi * 64
                        pos = slice(po, po + 64)
                        st = ps_stage.tile([128, NQ], F32,
                                           name=f"st{pair}_{kt}_{hi}",
                                           tag="stage")
                        for c in range(2):
                            cs = slice(c * 512, (c + 1) * 512)
                            nc.tensor.matmul(
                                st[:, cs], k_sb[pair][pos, kts],
                                q_sb[pair][pos, cs],
                                start=True, stop=True, tile_position=(po, 0))
                        praw = praw_p.tile([128, NQ], BF16,
                                           name=f"pr{pair}_{kt}_{hi}", tag="pr")
                        nc.scalar.activation(praw[:], st[:], AF.Exp)
                        phat = phat_p.tile([128, NQ], BF16,
                                           name=f"ph{pair}_{kt}_{hi}", tag="ph")
                        nc.vector.tensor_mul(phat[:], praw[:], em_sb[kt][:])
                        if dbg and pair == dbg_pair and kt == 0 and hi == 0:
                            dph = osb.tile([128, NQ], F32, name="dph", tag="ob")
                            nc.vector.tensor_copy(dph[:], phat[:])
                            nc.sync.dma_start(d_ph[:], dph[:])
                        # p@v col-packed into out_ps rows [po, po+64)
                        for c in range(2):
                            cs = slice(c * 512, (c + 1) * 512)
                            nc.tensor.matmul(
                                out_ps[pos, cs],
                                v_sb[kt][:, h * 64:(h + 1) * 64], phat[:, cs],
                                start=(kt == 0),
                                stop=(kt == NKT - 1),
                                skip_group_check=(hi == 1),
                                tile_position=(0, po))
                        # sums: 8 N=1 matmuls into the open group
                        for m in range(8):
                            col = hi * 8 + m
                            nc.tensor.matmul(
                                sums_ps[:, col:col + 1],
                                phat[:, m * 128:(m + 1) * 128], ones_bf[:],
                                start=False,
                                stop=(kt == NKT - 1 and hi == 1 and m == 7),
                                skip_group_check=True)

                # normalize: r = 1/sums; broadcast along free dim via
                # PE transpose + rank-1 matmuls
                sums_sb = small.tile([128, 16], F32, name=f"ss{pair}", tag="ss")
                nc.vector.tensor_copy(sums_sb[:], sums_ps[:, 0:16])
                if dbg and pair == dbg_pair:
                    nc.sync.dma_start(d_sum[:], sums_sb[:])
                    du = osb.tile([128, NQ], F32, name="du", tag="ob")
                    nc.vector.tensor_copy(du[:], out_ps[:])
                    nc.sync.dma_start(d_u[:], du[:])
                rrec = small.tile([128, 16], F32, name=f"rr{pair}", tag="rr")
                nc.vector.reciprocal(rrec[:], sums_sb[:])
                rT_ps = ps_misc.tile([16, 128], F32, name=f"rt{pair}",
                                     tag="misc")
                nc.tensor.transpose(rT_ps[:], rrec[:], ident[:])
                rT_sb = small2.tile([16, 128], F32R, name=f"rs{pair}",
                                    tag="rs")
                nc.vector.tensor_copy(rT_sb[:], rT_ps[:])
                # gather per-head rows: partition hi <- heads' 8 m-slices
                rrow = small2.tile([2, 8 * 128], F32R, name=f"rw{pair}",
                                   tag="rw")
                for j in range(16):
                    hi, m = j // 8, j % 8
                    nc.sync.dma_start(rrow[hi:hi + 1, m * 128:(m + 1) * 128],
                                      rT_sb[j:j + 1, :])
                if dbg and pair == dbg_pair:
                    nc.sync.dma_start(d_rr[:], rrow[:].bitcast(F32))
                for c in range(2):
                    cs = slice(c * 512, (c + 1) * 512)
                    bc_ps = ps_misc.tile([128, 512], F32,
                                         name=f"bc{pair}_{c}", tag="misc")
                    nc.tensor.matmul(bc_ps[:], ind[:], rrow[0:2, cs],
                                     start=True, stop=True)
                    bc_sb = small.tile([128, 512], F32, name=f"bs{pair}_{c}",
                                       tag="bs")
                    nc.vector.tensor_copy(bc_sb[:], bc_ps[:])
                    if dbg and pair == dbg_pair and c == 0:
                        nc.sync.dma_start(d_bc[:], bc_sb[:])
                    nc.vector.tensor_mul(uhat[pair][:, cs], out_ps[:, cs],
                                         bc_sb[:])
                if dbg:
                    dd = [d_uh, d_uh1, d_uh2, d_uh3][pair]
                    nc.sync.dma_start(dd[:], uhat[pair][:].bitcast(F32))

        # ---- phase 3: output projection ----
        for m in range(8):
            ms = slice(m * 128, (m + 1) * 128)
            pp = ps_stage.tile([128, DIM], F32, name=f"pp{m}", tag="stage")
            for kc in range(4):
                nc.tensor.matmul(pp[:], uhat[kc][:, ms], pj_sb[kc][:],
                                 start=(kc == 0), stop=(kc == 3))
            ob = osb.tile([128, DIM], F32, name=f"ob{m}", tag="ob")
            nc.vector.tensor_add(ob[:], pp[:], bias_sb[:])
            nc.sync.dma_start(out[ms, :], ob[:])

    nc.compile()
    return nc


@functools.lru_cache(maxsize=1)
def _get_nc():
    return build()


def _prep_inputs(x, attn_mask, qkv_w, proj_w, proj_b):
    x = np.asarray(x, dtype=np.float32)
    mask = np.asarray(attn_mask, dtype=np.float32).reshape(N, N)
    qkv_w = np.asarray(qkv_w, dtype=np.float32)
    proj_w = np.asarray(proj_w, dtype=np.float32)
    proj_b = np.asarray(proj_b, dtype=np.float32)

    wqT = np.ascontiguousarray((qkv_w[0:DIM] * SCALE).T)
    wkT = np.ascontiguousarray(qkv_w[DIM:2 * DIM].T)
    wvT = np.ascontiguousarray(qkv_w[2 * DIM:3 * DIM].T)
    projT = np.ascontiguousarray(proj_w.T)
    biasb = np.tile(proj_b, (128, 1))
    indmat = np.zeros((2, 128), dtype=np.float32)
    indmat[0, 0:64] = 1.0
    indmat[1, 64:128] = 1.0

    expm = np.exp(mask)
    # per-core key permutation: own seq-half first, other half second, so
    # the query chunk is always columns [0, NQ) of the permuted x.T
    xTs = {}
    emTs = {}
    for s in range(2):
        o = 1 - s
        emT = np.ascontiguousarray(expm[s * NQ:(s + 1) * NQ, :].T)  # [keys, q]
        emTs[s] = np.concatenate(
            [emT[s * NQ:(s + 1) * NQ], emT[o * NQ:(o + 1) * NQ]], axis=0
        ).astype(ml_dtypes.bfloat16)
        for b in range(B):
            xT = x[b].T  # [DIM, N]
            xTs[(b, s)] = np.ascontiguousarray(np.concatenate(
                [xT[:, s * NQ:(s + 1) * NQ], xT[:, o * NQ:(o + 1) * NQ]],
                axis=1))

    in_maps = []
    for c in range(NCORES):
        b, s = c // 2, c % 2
        in_maps.append({
            "xT": xTs[(b, s)],
            "wqT": wqT, "wkT": wkT, "wvT": wvT, "projT": projT,
            "biasb": biasb, "expmT": emTs[s], "indD": indmat,
        })
    return in_maps


def run(inputs, trace=False, tmpdir=None):
    nc = _get_nc()
    in_maps = _prep_inputs(**inputs)
    res = run_bass_kernel_spmd(nc, in_maps, core_ids=list(range(NCORES)),
                               trace=trace, tmpdir=tmpdir)
    full = np.empty((B, N, DIM), dtype=np.float32)
    for c in range(NCORES):
        b, s = c // 2, c % 2
        full[b, s * NQ:(s + 1) * NQ, :] = res.results[c]["out"]
    return full, res


def kernel(**inputs) -> np.ndarray:
    return run(inputs)[0]


# revision 13
# speedup vs baseline: 1.0271x; 1.0271x over previous
"""Trainium2 Bass kernel for nn_Attention (B=4, N=2048, DIM=512, H=8).

Sharding: 8 cores = (batch b, seq-half s). Each core computes attention
outputs for queries [s*1024, (s+1)*1024) of batch b, all 8 heads, plus
the output projection for those rows. Outputs are disjoint -> host
gather is a pure concatenation (no reduction).

Per-core dataflow (all layouts chosen so no on-device transposes of
large tensors are ever needed):
  q_T [512,1024]  = (SCALE*wq) @ x_chunk.T    (features x queries)
  k_T [512,2048]  = wk @ x.T                  (features x keys)
  v   [2048, 512] = x @ wv.T                  (keys x features)
  per head h: scores_T[k,q] = k_h @ q_h.T     (keys on partitions)
       p = exp(scores_T) * exp(mask).T        (mask add via exp-multiply)
       U.T[d,q] += v_h.T @ p  (PSUM accum over key tiles, heads col-packed)
       sums[q] += ones.T @ p  (N=1 matmuls, queries on partitions)
  Uhat = U * (1/sums broadcast via transpose + rank-1 matmul)
  out[q,:] = Uhat.T @ proj_w.T + bias
"""
import functools
import numpy as np
import ml_dtypes
from contextlib import ExitStack

import concourse.bass as bass
import concourse.tile as tile
from concourse import bacc, mybir
from concourse.bass_utils import run_bass_kernel_spmd
from concourse.masks import make_identity

F32 = mybir.dt.float32
F32R = mybir.dt.float32r
BF16 = mybir.dt.bfloat16
AF = mybir.ActivationFunctionType

B, N, DIM, H, D = 4, 2048, 512, 8, 64
SCALE = D ** -0.5
NQ = N // 2          # queries per core
NKT = N // 128       # key tiles (16)
NCORES = 8


def build(dbg=False, dbg_pair=0):
    nc = bacc.Bacc("TRN2", target_bir_lowering=False, debug=False,
                   num_devices=NCORES)
    xT = nc.dram_tensor("xT", [DIM, N], F32R, kind="ExternalInput").ap()
    wqT = nc.dram_tensor("wqT", [DIM, DIM], F32R, kind="ExternalInput").ap()
    wkT = nc.dram_tensor("wkT", [DIM, DIM], F32R, kind="ExternalInput").ap()
    wvT = nc.dram_tensor("wvT", [DIM, DIM], F32R, kind="ExternalInput").ap()
    projT = nc.dram_tensor("projT", [DIM, DIM], F32R, kind="ExternalInput").ap()
    biasb = nc.dram_tensor("biasb", [128, DIM], F32, kind="ExternalInput").ap()
    expmT = nc.dram_tensor("expmT", [N, NQ], BF16, kind="ExternalInput").ap()
    indD = nc.dram_tensor("indD", [2, 128], F32R, kind="ExternalInput").ap()
    out = nc.dram_tensor("out", [NQ, DIM], F32, kind="ExternalOutput").ap()
    if dbg:
        d_q = nc.dram_tensor("d_q", [128, NQ], F32, kind="ExternalOutput").ap()
        d_k = nc.dram_tensor("d_k", [128, N], F32, kind="ExternalOutput").ap()
        d_v = nc.dram_tensor("d_v", [128, DIM], F32, kind="ExternalOutput").ap()
        d_ph = nc.dram_tensor("d_ph", [128, NQ], F32, kind="ExternalOutput").ap()
        d_sum = nc.dram_tensor("d_sum", [128, 16], F32, kind="ExternalOutput").ap()
        d_rr = nc.dram_tensor("d_rr", [2, 1024], F32, kind="ExternalOutput").ap()
        d_bc = nc.dram_tensor("d_bc", [128, 512], F32, kind="ExternalOutput").ap()
        d_u = nc.dram_tensor("d_u", [128, NQ], F32, kind="ExternalOutput").ap()
        d_uh = nc.dram_tensor("d_uh", [128, NQ], F32, kind="ExternalOutput").ap()
        d_uh1 = nc.dram_tensor("d_uh1", [128, NQ], F32, kind="ExternalOutput").ap()
        d_uh2 = nc.dram_tensor("d_uh2", [128, NQ], F32, kind="ExternalOutput").ap()
        d_uh3 = nc.dram_tensor("d_uh3", [128, NQ], F32, kind="ExternalOutput").ap()

    with tile.TileContext(nc) as tc, ExitStack() as ctx:
        # ---- pools ----
        wp = ctx.enter_context(tc.tile_pool(name="wp", bufs=1))
        kv = ctx.enter_context(tc.tile_pool(name="kv", bufs=1))
        small = ctx.enter_context(tc.tile_pool(name="small", bufs=2))
        small2 = ctx.enter_context(tc.tile_pool(name="small2", bufs=1))
        osb = ctx.enter_context(tc.tile_pool(name="osb", bufs=2))
        ps_stage = ctx.enter_context(tc.tile_pool(name="ps_stage", bufs=2, space="PSUM"))
        ps_out = ctx.enter_context(tc.tile_pool(name="ps_out", bufs=1, space="PSUM"))
        ps_misc = ctx.enter_context(tc.tile_pool(name="ps_misc", bufs=2, space="PSUM"))

        # ---- constants ----
        ident = wp.tile([128, 128], F32, name="ident", tag="ident")
        make_identity(nc, ident[:])
        ones_bf = wp.tile([128, 1], BF16, name="ones_bf", tag="ones_bf")
        nc.vector.memset(ones_bf[:], 1.0)
        zcol_bf = wp.tile([1, 128], BF16, name="zcol_bf", tag="zcol_bf")
        nc.vector.memset(zcol_bf[:], 0.0)
        zrow_bf = wp.tile([1, 512], BF16, name="zrow_bf", tag="zrow_bf")
        nc.vector.memset(zrow_bf[:], 1.0)
        ind = wp.tile([2, 128], F32R, name="ind", tag="ind")
        nc.sync.dma_start(ind[:], indD[:])

        # ---- weight / persistent loads ----
        pj_sb = [wp.tile([128, DIM], F32R, name=f"pj{kc}", tag=f"pj{kc}") for kc in range(4)]
        for kc in range(4):
            sl = slice(kc * 128, (kc + 1) * 128)
            nc.sync.dma_start(pj_sb[kc][:], projT[sl, :])
        bias_sb = wp.tile([128, DIM], F32, name="bias_sb", tag="bias_sb")
        nc.sync.dma_start(bias_sb[:], biasb[:])

        q_sb = [kv.tile([128, NQ], F32R, name=f"q{m}", tag=f"q{m}") for m in range(4)]
        k_sb = [kv.tile([128, N], F32R, name=f"k{m}", tag=f"k{m}") for m in range(4)]
        v_sb = [kv.tile([128, DIM], BF16, name=f"v{kt}", tag=f"v{kt}") for kt in range(NKT)]
        em_sb = [kv.tile([128, NQ], BF16, name=f"em{kt}", tag=f"em{kt}") for kt in range(NKT)]
        for kt in range(NKT):
            nc.sync.dma_start(em_sb[kt][:], expmT[kt * 128:(kt + 1) * 128, :])
        uhat = [kv.tile([128, NQ], F32R, name=f"uh{p}", tag=f"uh{p}") for p in range(4)]

        # ---- phase 1: projections ----
        with tc.tile_pool(name="xp", bufs=1) as xp:
            x_sb = [xp.tile([128, N], F32R, name=f"x{kc}", tag=f"x{kc}") for kc in range(4)]
            wq_sb = [xp.tile([128, DIM], F32R, name=f"wq{kc}", tag=f"wq{kc}") for kc in range(4)]
            wk_sb = [xp.tile([128, DIM], F32R, name=f"wk{kc}", tag=f"wk{kc}") for kc in range(4)]
            wv_sb = [xp.tile([128, DIM], F32R, name=f"wv{kc}", tag=f"wv{kc}") for kc in range(4)]
            for kc in range(4):
                sl = slice(kc * 128, (kc + 1) * 128)
                nc.sync.dma_start(x_sb[kc][:], xT[sl, :])
                nc.sync.dma_start(wq_sb[kc][:], wqT[sl, :])
                nc.sync.dma_start(wk_sb[kc][:], wkT[sl, :])
                nc.sync.dma_start(wv_sb[kc][:], wvT[sl, :])

            # q_T: [512, 1024] by feature tile m
            for m in range(4):
                ms = slice(m * 128, (m + 1) * 128)
                ps = ps_stage.tile([128, NQ], F32, name=f"psq{m}", tag="stage")
                for c in range(2):
                    cs = slice(c * 512, (c + 1) * 512)
                    for kc in range(4):
                        nc.tensor.matmul(ps[:, cs], wq_sb[kc][:, ms],
                                         x_sb[kc][:, cs],
                                         start=(kc == 0), stop=(kc == 3))
                nc.vector.tensor_copy(q_sb[m][:], ps[:])

            # k_T: [512, 2048]
            for m in range(4):
                ms = slice(m * 128, (m + 1) * 128)
                for half in range(2):
                    ps = ps_stage.tile([128, NQ], F32, name=f"psk{m}_{half}",
                                       tag="stage")
                    for c2 in range(2):
                        cs_o = slice(c2 * 512, (c2 + 1) * 512)
                        cs_x = slice(half * 1024 + c2 * 512,
                                     half * 1024 + (c2 + 1) * 512)
                        for kc in range(4):
                            nc.tensor.matmul(ps[:, cs_o], wk_sb[kc][:, ms],
                                             x_sb[kc][:, cs_x],
                                             start=(kc == 0), stop=(kc == 3))
                    nc.vector.tensor_copy(
                        k_sb[m][:, half * 1024:(half + 1) * 1024], ps[:])

            # v: [2048, 512] natural
            for kt in range(NKT):
                ks = slice(kt * 128, (kt + 1) * 128)
                ps = ps_misc.tile([128, DIM], F32, name=f"psv{kt}", tag="misc")
                for kc in range(4):
                    nc.tensor.matmul(ps[:], x_sb[kc][:, ks], wv_sb[kc][:],
                                     start=(kc == 0), stop=(kc == 3))
                nc.vector.tensor_copy(v_sb[kt][:], ps[:])

        if dbg:
            nc.sync.dma_start(d_q[:], q_sb[0][:].bitcast(F32))
            nc.sync.dma_start(d_k[:], k_sb[0][:].bitcast(F32))
            dvf = osb.tile([128, DIM], F32, name="dvf", tag="ob")
            nc.vector.tensor_copy(dvf[:], v_sb[0][:])
            nc.sync.dma_start(d_v[:], dvf[:])

        # ---- phase 2: attention per head-pair ----
        with tc.tile_pool(name="praw", bufs=4) as praw_p, \
             tc.tile_pool(name="phat", bufs=8) as phat_p:
            for pair in range(4):
                out_ps = ps_out.tile([128, NQ], F32, name=f"o{pair}", tag="out")
                sums_ps = ps_misc.tile([128, 512], F32, name=f"s{pair}",
                                       tag="misc")
                # open ONE accumulation group over the sums bank (zeros it)
                nc.tensor.matmul(sums_ps[:], zcol_bf[:], zrow_bf[:],
                                 start=True, stop=False)
                for kt in range(NKT):
                    kts = slice(kt * 128, (kt + 1) * 128)
                    sts, praws, phats = [], [], []
                    # scores: both heads emitted adjacently so the K=64
                    # row-packed pairs issue concurrently on the PE
                    for hi in range(2):
                        sts.append(ps_stage.tile(
                            [128, NQ], F32, name=f"st{pair}_{kt}_{hi}",
                            tag="stage"))
                    for c in range(2):
                        cs = slice(c * 512, (c + 1) * 512)
                        for hi in range(2):
                            po = hi * 64
                            pos = slice(po, po + 64)
                            nc.tensor.matmul(
                                sts[hi][:, cs], k_sb[pair][pos, kts],
                                q_sb[pair][pos, cs],
                                start=True, stop=True, tile_position=(po, 0))
                    for hi in range(2):
                        praw = praw_p.tile([128, NQ], BF16,
                                           name=f"pr{pair}_{kt}_{hi}", tag="pr")
                        nc.scalar.activation(praw[:], sts[hi][:], AF.Exp)
                        praws.append(praw)
                    for hi in range(2):
                        phat = phat_p.tile([128, NQ], BF16,
                                           name=f"ph{pair}_{kt}_{hi}", tag="ph")
                        nc.vector.tensor_mul(phat[:], praws[hi][:], em_sb[kt][:])
                        phats.append(phat)
                    if dbg and pair == dbg_pair and kt == 0:
                        dph = osb.tile([128, NQ], F32, name="dph", tag="ob")
                        nc.vector.tensor_copy(dph[:], phats[0][:])
                        nc.sync.dma_start(d_ph[:], dph[:])
                    # p@v: col-packed pairs adjacent
                    for c in range(2):
                        cs = slice(c * 512, (c + 1) * 512)
                        for hi in range(2):
                            h = 2 * pair + hi
                            po = hi * 64
                            pos = slice(po, po + 64)
                            nc.tensor.matmul(
                                out_ps[pos, cs],
                                v_sb[kt][:, h * 64:(h + 1) * 64],
                                phats[hi][:, cs],
                                start=(kt == 0),
                                stop=(kt == NKT - 1),
                                skip_group_check=(hi == 1),
                                tile_position=(0, po))
                    # sums: 16 N=1 matmuls into the open group
                    for hi in range(2):
                        for m in range(8):
                            col = hi * 8 + m
                            nc.tensor.matmul(
                                sums_ps[:, col:col + 1],
                                phats[hi][:, m * 128:(m + 1) * 128], ones_bf[:],
                                start=False,
                                stop=(kt == NKT - 1 and hi == 1 and m == 7),
                                skip_group_check=True)

                # normalize: r = 1/sums; broadcast along free dim via
                # PE transpose + rank-1 matmuls
                sums_sb = small.tile([128, 16], F32, name=f"ss{pair}", tag="ss")
                nc.vector.tensor_copy(sums_sb[:], sums_ps[:, 0:16])
                if dbg and pair == dbg_pair:
                    nc.sync.dma_start(d_sum[:], sums_sb[:])
                    du = osb.tile([128, NQ], F32, name="du", tag="ob")
                    nc.vector.tensor_copy(du[:], out_ps[:])
                    nc.sync.dma_start(d_u[:], du[:])
                rrec = small.tile([128, 16], F32, name=f"rr{pair}", tag="rr")
                nc.vector.reciprocal(rrec[:], sums_sb[:])
                rT_ps = ps_misc.tile([16, 128], F32, name=f"rt{pair}",
                                     tag="misc")
                nc.tensor.transpose(rT_ps[:], rrec[:], ident[:])
                rT_sb = small2.tile([16, 128], F32R, name=f"rs{pair}",
                                    tag="rs")
                nc.vector.tensor_copy(rT_sb[:], rT_ps[:])
                # gather per-head rows: partition hi <- heads' 8 m-slices
                rrow = small2.tile([2, 8 * 128], F32R, name=f"rw{pair}",
                                   tag="rw")
                for j in range(16):
                    hi, m = j // 8, j % 8
                    nc.sync.dma_start(rrow[hi:hi + 1, m * 128:(m + 1) * 128],
                                      rT_sb[j:j + 1, :])
                if dbg and pair == dbg_pair:
                    nc.sync.dma_start(d_rr[:], rrow[:].bitcast(F32))
                for c in range(2):
                    cs = slice(c * 512, (c + 1) * 512)
                    bc_ps = ps_misc.tile([128, 512], F32,
                                         name=f"bc{pair}_{c}", tag="misc")
                    nc.tensor.matmul(bc_ps[:], ind[:], rrow[0:2, cs],
                                     start=True, stop=True)
                    bc_sb = small.tile([128, 512], F32, name=f"bs{pair}_{c}",
                                       tag="bs")
                    nc.vector.tensor_copy(bc_sb[:], bc_ps[:])
                    if dbg and pair == dbg_pair and c == 0:
                        nc.sync.dma_start(d_bc[:], bc_sb[:])
                    nc.vector.tensor_mul(uhat[pair][:, cs], out_ps[:, cs],
                                         bc_sb[:])
                if dbg:
                    dd = [d_uh, d_uh1, d_uh2, d_uh3][pair]
                    nc.sync.dma_start(dd[:], uhat[pair][:].bitcast(F32))

        # ---- phase 3: output projection ----
        for m in range(8):
            ms = slice(m * 128, (m + 1) * 128)
            pp = ps_stage.tile([128, DIM], F32, name=f"pp{m}", tag="stage")
            for kc in range(4):
                nc.tensor.matmul(pp[:], uhat[kc][:, ms], pj_sb[kc][:],
                                 start=(kc == 0), stop=(kc == 3))
            ob = osb.tile([128, DIM], F32, name=f"ob{m}", tag="ob")
            nc.vector.tensor_add(ob[:], pp[:], bias_sb[:])
            nc.sync.dma_start(out[ms, :], ob[:])

    nc.compile()
    return nc


@functools.lru_cache(maxsize=1)
def _get_nc():
    return build()


def _prep_inputs(x, attn_mask, qkv_w, proj_w, proj_b):
    x = np.asarray(x, dtype=np.float32)
    mask = np.asarray(attn_mask, dtype=np.float32).reshape(N, N)
    qkv_w = np.asarray(qkv_w, dtype=np.float32)
    proj_w = np.asarray(proj_w, dtype=np.float32)
    proj_b = np.asarray(proj_b, dtype=np.float32)

    wqT = np.ascontiguousarray((qkv_w[0:DIM] * SCALE).T)
    wkT = np.ascontiguousarray(qkv_w[DIM:2 * DIM].T)
    wvT = np.ascontiguousarray(qkv_w[2 * DIM:3 * DIM].T)
    projT = np.ascontiguousarray(proj_w.T)
    biasb = np.tile(proj_b, (128, 1))
    indmat = np.zeros((2, 128), dtype=np.float32)
    indmat[0, 0:64] = 1.0
    indmat[1, 64:128] = 1.0

    expm = np.exp(mask)
    # per-core key permutation: own seq-half first, other half second, so
    # the query chunk is always columns [0, NQ) of the permuted x.T
    xTs = {}
    emTs = {}
    for s in range(2):
        o = 1 - s
        emT = np.ascontiguousarray(expm[s * NQ:(s + 1) * NQ, :].T)  # [keys, q]
        emTs[s] = np.concatenate(
            [emT[s * NQ:(s + 1) * NQ], emT[o * NQ:(o + 1) * NQ]], axis=0
        ).astype(ml_dtypes.bfloat16)
        for b in range(B):
            xT = x[b].T  # [DIM, N]
            xTs[(b, s)] = np.ascontiguousarray(np.concatenate(
                [xT[:, s * NQ:(s + 1) * NQ], xT[:, o * NQ:(o + 1) * NQ]],
                axis=1))

    in_maps = []
    for c in range(NCORES):
        b, s = c // 2, c % 2
        in_maps.append({
            "xT": xTs[(b, s)],
            "wqT": wqT, "wkT": wkT, "wvT": wvT, "projT": projT,
            "biasb": biasb, "expmT": emTs[s], "indD": indmat,
        })
    return in_maps


def run(inputs, trace=False, tmpdir=None):
    nc = _get_nc()
    in_maps = _prep_inputs(**inputs)
    res = run_bass_kernel_spmd(nc, in_maps, core_ids=list(range(NCORES)),
                               trace=trace, tmpdir=tmpdir)
    full = np.empty((B, N, DIM), dtype=np.float32)
    for c in range(NCORES):
        b, s = c // 2, c % 2
        full[b, s * NQ:(s + 1) * NQ, :] = res.results[c]["out"]
    return full, res


def kernel(**inputs) -> np.ndarray:
    return run(inputs)[0]


# revision 15
# speedup vs baseline: 1.2025x; 1.1708x over previous
"""Trainium2 Bass kernel for nn_Attention (B=4, N=2048, DIM=512, H=8).

Sharding: 8 cores = (batch b, seq-half s). Each core computes attention
outputs for queries [s*1024, (s+1)*1024) of batch b, all 8 heads, plus
the output projection for those rows. Outputs are disjoint -> host
gather is a pure concatenation (no reduction). Keys are permuted per
core (own seq-half first) so the query chunk is always columns [0, NQ)
of the permuted x.T; attention is permutation-invariant over keys.

Per-core dataflow (layouts chosen so no on-device transposes of large
tensors are needed):
  q_T [512,1024]  = (SCALE*wq) @ x_chunk.T    (features x queries)
  k_T [512,2048]  = wk @ x.T                  (features x keys)
  v   [2048, 512] = x @ wv.T                  (keys x features)
  per head-pair, seq-half, key-tile kt:
       scores_T[k, (h0 q | h1 q)] = k_h @ q_h.T   (K=64 row-packed pair)
       p = exp(scores_T) * exp(mask).T         (mask add via exp-multiply)
       U.T[d,q] += v_h.T @ p   (PSUM accum over kt, heads col-packed M=64)
       sums[q]  += p.T @ ones  (N=1 matmuls, queries on partitions)
  Uhat = U * (1/sums broadcast via PE transpose + K=2 indicator matmul)
  out[q,:] = Uhat.T @ proj_w.T + bias
"""
import functools
import numpy as np
import ml_dtypes
from contextlib import ExitStack

import concourse.bass as bass
import concourse.tile as tile
from concourse import bacc, mybir
from concourse.bass_utils import run_bass_kernel_spmd
from concourse.masks import make_identity

F32 = mybir.dt.float32
F32R = mybir.dt.float32r
BF16 = mybir.dt.bfloat16
AF = mybir.ActivationFunctionType

B, N, DIM, H, D = 4, 2048, 512, 8, 64
SCALE = D ** -0.5
NQ = N // 2          # queries per core
NKT = N // 128       # key tiles (16)
NCORES = 8


def build(dbg=False):
    nc = bacc.Bacc("TRN2", target_bir_lowering=False, debug=False,
                   num_devices=NCORES)
    xT = nc.dram_tensor("xT", [DIM, N], F32R, kind="ExternalInput").ap()
    wqT = nc.dram_tensor("wqT", [DIM, DIM], F32R, kind="ExternalInput").ap()
    wkT = nc.dram_tensor("wkT", [DIM, DIM], F32R, kind="ExternalInput").ap()
    wvT = nc.dram_tensor("wvT", [DIM, DIM], F32R, kind="ExternalInput").ap()
    projT = nc.dram_tensor("projT", [DIM, DIM], F32R, kind="ExternalInput").ap()
    biasb = nc.dram_tensor("biasb", [128, DIM], F32, kind="ExternalInput").ap()
    expmT = nc.dram_tensor("expmT", [N, NQ], BF16, kind="ExternalInput").ap()
    indD = nc.dram_tensor("indD", [2, 128], F32R, kind="ExternalInput").ap()
    out = nc.dram_tensor("out", [NQ, DIM], F32, kind="ExternalOutput").ap()

    with tile.TileContext(nc) as tc, ExitStack() as ctx:
        # ---- SBUF pools ----
        wp = ctx.enter_context(tc.tile_pool(name="wp", bufs=1))
        kv = ctx.enter_context(tc.tile_pool(name="kv", bufs=1))
        small = ctx.enter_context(tc.tile_pool(name="small", bufs=2))
        osb = ctx.enter_context(tc.tile_pool(name="osb", bufs=2))
        # ---- PSUM pools: 4 + 2 + 1 + 1 = 8 banks ----
        ps_stage = ctx.enter_context(
            tc.tile_pool(name="ps_stage", bufs=2, space="PSUM"))   # 2x2 banks
        ps_out = ctx.enter_context(
            tc.tile_pool(name="ps_out", bufs=2, space="PSUM"))     # 2x1 bank
        ps_sums = ctx.enter_context(
            tc.tile_pool(name="ps_sums", bufs=1, space="PSUM"))    # 1 bank
        ps_misc = ctx.enter_context(
            tc.tile_pool(name="ps_misc", bufs=1, space="PSUM"))    # 1 bank

        # ---- constants ----
        ident = wp.tile([128, 128], F32, name="ident", tag="ident")
        make_identity(nc, ident[:])
        ones_bf = wp.tile([128, 1], BF16, name="ones_bf", tag="ones_bf")
        nc.vector.memset(ones_bf[:], 1.0)
        zcol_bf = wp.tile([1, 128], BF16, name="zcol_bf", tag="zcol_bf")
        nc.vector.memset(zcol_bf[:], 0.0)
        zrow_bf = wp.tile([1, 512], BF16, name="zrow_bf", tag="zrow_bf")
        nc.vector.memset(zrow_bf[:], 1.0)
        ind = wp.tile([2, 128], F32R, name="ind", tag="ind")
        nc.sync.dma_start(ind[:], indD[:])

        # ---- persistent loads ----
        pj_sb = [wp.tile([128, DIM], F32R, name=f"pj{kc}", tag=f"pj{kc}")
                 for kc in range(4)]
        for kc in range(4):
            nc.sync.dma_start(pj_sb[kc][:], projT[kc * 128:(kc + 1) * 128, :])
        bias_sb = wp.tile([128, DIM], F32, name="bias_sb", tag="bias_sb")
        nc.sync.dma_start(bias_sb[:], biasb[:])

        q_sb = [kv.tile([128, NQ], F32R, name=f"q{m}", tag=f"q{m}")
                for m in range(4)]
        k_sb = [kv.tile([128, N], F32R, name=f"k{m}", tag=f"k{m}")
                for m in range(4)]
        v_sb = [kv.tile([128, DIM], BF16, name=f"v{kt}", tag=f"v{kt}")
                for kt in range(NKT)]
        em_sb = [kv.tile([128, NQ], BF16, name=f"em{kt}", tag=f"em{kt}")
                 for kt in range(NKT)]
        for kt in range(NKT):
            nc.sync.dma_start(em_sb[kt][:], expmT[kt * 128:(kt + 1) * 128, :])
        uhat = [kv.tile([128, NQ], F32R, name=f"uh{p}", tag=f"uh{p}")
                for p in range(4)]

        # ---- phase 1: projections ----
        with tc.tile_pool(name="xp", bufs=1) as xp:
            x_sb = [xp.tile([128, N], F32R, name=f"x{kc}", tag=f"x{kc}")
                    for kc in range(4)]
            wq_sb = [xp.tile([128, DIM], F32R, name=f"wq{kc}", tag=f"wq{kc}")
                     for kc in range(4)]
            wk_sb = [xp.tile([128, DIM], F32R, name=f"wk{kc}", tag=f"wk{kc}")
                     for kc in range(4)]
            wv_sb = [xp.tile([128, DIM], F32R, name=f"wv{kc}", tag=f"wv{kc}")
                     for kc in range(4)]
            for kc in range(4):
                sl = slice(kc * 128, (kc + 1) * 128)
                nc.sync.dma_start(x_sb[kc][:], xT[sl, :])
                nc.sync.dma_start(wq_sb[kc][:], wqT[sl, :])
                nc.sync.dma_start(wk_sb[kc][:], wkT[sl, :])
                nc.sync.dma_start(wv_sb[kc][:], wvT[sl, :])

            # q_T [512, 1024] (queries = cols 0:NQ of permuted xT)
            for m in range(4):
                ms = slice(m * 128, (m + 1) * 128)
                ps = ps_stage.tile([128, NQ], F32, name=f"psq{m}", tag="stage")
                for c in range(2):
                    cs = slice(c * 512, (c + 1) * 512)
                    for kc in range(4):
                        nc.tensor.matmul(ps[:, cs], wq_sb[kc][:, ms],
                                         x_sb[kc][:, cs],
                                         start=(kc == 0), stop=(kc == 3))
                nc.vector.tensor_copy(q_sb[m][:], ps[:])

            # k_T [512, 2048]
            for m in range(4):
                ms = slice(m * 128, (m + 1) * 128)
                for half in range(2):
                    ps = ps_stage.tile([128, NQ], F32, name=f"psk{m}_{half}",
                                       tag="stage")
                    for c2 in range(2):
                        cs_o = slice(c2 * 512, (c2 + 1) * 512)
                        cs_x = slice(half * 1024 + c2 * 512,
                                     half * 1024 + (c2 + 1) * 512)
                        for kc in range(4):
                            nc.tensor.matmul(ps[:, cs_o], wk_sb[kc][:, ms],
                                             x_sb[kc][:, cs_x],
                                             start=(kc == 0), stop=(kc == 3))
                    nc.vector.tensor_copy(
                        k_sb[m][:, half * 1024:(half + 1) * 1024], ps[:])

            # v [2048, 512]
            for kt in range(NKT):
                ks = slice(kt * 128, (kt + 1) * 128)
                ps = ps_out.tile([128, DIM], F32, name=f"psv{kt}", tag="out")
                for kc in range(4):
                    nc.tensor.matmul(ps[:], x_sb[kc][:, ks], wv_sb[kc][:],
                                     start=(kc == 0), stop=(kc == 3))
                nc.vector.tensor_copy(v_sb[kt][:], ps[:])

        # ---- phase 2: attention ----
        with tc.tile_pool(name="praw", bufs=3) as praw_p, \
             tc.tile_pool(name="phat", bufs=6) as phat_p:
            for pair in range(4):
                for half in range(2):
                    hq = slice(half * 512, (half + 1) * 512)
                    out_ps = ps_out.tile([128, 512], F32,
                                         name=f"o{pair}_{half}", tag="out")
                    sums_ps = ps_sums.tile([128, 8], F32,
                                           name=f"s{pair}_{half}", tag="sums")
                    # open ONE accumulation group over the sums bank
                    nc.tensor.matmul(sums_ps[:], zcol_bf[:], zrow_bf[:, 0:8],
                                     start=True, stop=False)
                    for kt in range(NKT):
                        kts = slice(kt * 128, (kt + 1) * 128)
                        st = ps_stage.tile([128, 1024], F32,
                                           name=f"st{pair}_{half}_{kt}",
                                           tag="stage")
                        # scores: both heads adjacent -> row-packed pair
                        for hi in range(2):
                            po = hi * 64
                            pos = slice(po, po + 64)
                            nc.tensor.matmul(
                                st[:, hi * 512:(hi + 1) * 512],
                                k_sb[pair][pos, kts], q_sb[pair][pos, hq],
                                start=True, stop=True, tile_position=(po, 0))
                        praw = praw_p.tile([128, 1024], BF16,
                                           name=f"pr{pair}_{half}_{kt}",
                                           tag="pr")
                        nc.scalar.activation(praw[:], st[:], AF.Exp)
                        phat = phat_p.tile([128, 1024], BF16,
                                           name=f"ph{pair}_{half}_{kt}",
                                           tag="ph")
                        em2 = em_sb[kt][:, hq].rearrange(
                            "p (o f) -> p o f", o=1).broadcast_to([128, 2, 512])
                        nc.vector.tensor_mul(
                            phat[:].rearrange("p (t f) -> p t f", t=2),
                            praw[:].rearrange("p (t f) -> p t f", t=2), em2)
                        # p@v: col-packed pair
                        for hi in range(2):
                            h = 2 * pair + hi
                            po = hi * 64
                            pos = slice(po, po + 64)
                            nc.tensor.matmul(
                                out_ps[pos, :],
                                v_sb[kt][:, h * 64:(h + 1) * 64],
                                phat[:, hi * 512:(hi + 1) * 512],
                                start=(kt == 0), stop=(kt == NKT - 1),
                                skip_group_check=(hi == 1),
                                tile_position=(0, po))
                        # sums: 8 N=1 matmuls
                        for col in range(8):
                            nc.tensor.matmul(
                                sums_ps[:, col:col + 1],
                                phat[:, col * 128:(col + 1) * 128],
                                ones_bf[:],
                                start=False,
                                stop=(kt == NKT - 1 and col == 7),
                                skip_group_check=True)

                    # normalize: r = 1/sums broadcast along the free dim
                    sums_sb = small.tile([128, 8], F32,
                                         name=f"ss{pair}_{half}", tag="ss")
                    nc.vector.tensor_copy(sums_sb[:], sums_ps[:])
                    rrec = small.tile([128, 8], F32,
                                      name=f"rr{pair}_{half}", tag="rr")
                    nc.vector.reciprocal(rrec[:], sums_sb[:])
                    rT_ps = ps_misc.tile([8, 128], F32,
                                         name=f"rt{pair}_{half}", tag="misc")
                    nc.tensor.transpose(rT_ps[:], rrec[:], ident[:])
                    rT_sb = small.tile([8, 128], F32R,
                                       name=f"rs{pair}_{half}", tag="rs")
                    nc.vector.tensor_copy(rT_sb[:], rT_ps[:])
                    rrow = small.tile([2, 4 * 128], F32R,
                                      name=f"rw{pair}_{half}", tag="rw")
                    for j in range(8):
                        hi, m = j // 4, j % 4
                        nc.sync.dma_start(
                            rrow[hi:hi + 1, m * 128:(m + 1) * 128],
                            rT_sb[j:j + 1, :])
                    bc_ps = ps_misc.tile([128, 512], F32,
                                         name=f"bc{pair}_{half}", tag="misc")
                    nc.tensor.matmul(bc_ps[:], ind[:], rrow[0:2, :],
                                     start=True, stop=True)
                    bc_sb = small.tile([128, 512], F32,
                                       name=f"bs{pair}_{half}", tag="bs")
                    nc.vector.tensor_copy(bc_sb[:], bc_ps[:])
                    nc.vector.tensor_mul(uhat[pair][:, hq], out_ps[:],
                                         bc_sb[:])

        # ---- phase 3: output projection ----
        for m in range(8):
            ms = slice(m * 128, (m + 1) * 128)
            pp = ps_stage.tile([128, DIM], F32, name=f"pp{m}", tag="stage")
            for kc in range(4):
                nc.tensor.matmul(pp[:], uhat[kc][:, ms], pj_sb[kc][:],
                                 start=(kc == 0), stop=(kc == 3))
            ob = osb.tile([128, DIM], F32, name=f"ob{m}", tag="ob")
            nc.vector.tensor_add(ob[:], pp[:], bias_sb[:])
            nc.sync.dma_start(out[ms, :], ob[:])

    nc.compile()
    return nc


@functools.lru_cache(maxsize=1)
def _get_nc():
    return build()


def _prep_inputs(x, attn_mask, qkv_w, proj_w, proj_b):
    x = np.asarray(x, dtype=np.float32)
    mask = np.asarray(attn_mask, dtype=np.float32).reshape(N, N)
    qkv_w = np.asarray(qkv_w, dtype=np.float32)
    proj_w = np.asarray(proj_w, dtype=np.float32)
    proj_b = np.asarray(proj_b, dtype=np.float32)

    wqT = np.ascontiguousarray((qkv_w[0:DIM] * SCALE).T)
    wkT = np.ascontiguousarray(qkv_w[DIM:2 * DIM].T)
    wvT = np.ascontiguousarray(qkv_w[2 * DIM:3 * DIM].T)
    projT = np.ascontiguousarray(proj_w.T)
    biasb = np.tile(proj_b, (128, 1))
    indmat = np.zeros((2, 128), dtype=np.float32)
    indmat[0, 0:64] = 1.0
    indmat[1, 64:128] = 1.0

    expm = np.exp(mask)
    # per-core key permutation: own seq-half first, other half second, so
    # the query chunk is always columns [0, NQ) of the permuted x.T
    xTs = {}
    emTs = {}
    for s in range(2):
        o = 1 - s
        emT = np.ascontiguousarray(expm[s * NQ:(s + 1) * NQ, :].T)  # [keys, q]
        emTs[s] = np.concatenate(
            [emT[s * NQ:(s + 1) * NQ], emT[o * NQ:(o + 1) * NQ]], axis=0
        ).astype(ml_dtypes.bfloat16)
        for b in range(B):
            xTb = x[b].T  # [DIM, N]
            xTs[(b, s)] = np.ascontiguousarray(np.concatenate(
                [xTb[:, s * NQ:(s + 1) * NQ], xTb[:, o * NQ:(o + 1) * NQ]],
                axis=1))

    in_maps = []
    for c in range(NCORES):
        b, s = c // 2, c % 2
        in_maps.append({
            "xT": xTs[(b, s)],
            "wqT": wqT, "wkT": wkT, "wvT": wvT, "projT": projT,
            "biasb": biasb, "expmT": emTs[s], "indD": indmat,
        })
    return in_maps


def run(inputs, trace=False, tmpdir=None):
    nc = _get_nc()
    in_maps = _prep_inputs(**inputs)
    res = run_bass_kernel_spmd(nc, in_maps, core_ids=list(range(NCORES)),
                               trace=trace, tmpdir=tmpdir)
    full = np.empty((B, N, DIM), dtype=np.float32)
    for c in range(NCORES):
        b, s = c // 2, c % 2
        full[b, s * NQ:(s + 1) * NQ, :] = res.results[c]["out"]
    return full, res


def kernel(**inputs) -> np.ndarray:
    return run(inputs)[0]


# revision 17
# speedup vs baseline: 1.2641x; 1.0512x over previous
"""Trainium2 Bass kernel for nn_Attention (B=4, N=2048, DIM=512, H=8).

Sharding: 8 cores = (batch b, seq-half s). Each core computes attention
outputs for queries [s*1024, (s+1)*1024) of batch b, all 8 heads, plus
the output projection for those rows. Outputs are disjoint -> host
gather is a pure concatenation (no reduction). Keys are permuted per
core (own seq-half first) so the query chunk is always columns [0, NQ)
of the permuted x.T; attention is permutation-invariant over keys.

Per-core dataflow (layouts chosen so no on-device transposes of large
tensors are needed):
  q_T [512,1024]  = (SCALE*wq) @ x_chunk.T    (features x queries)
  k_T [512,2048]  = wk @ x.T                  (features x keys)
  v   [2048, 512] = x @ wv.T                  (keys x features)
  per head-pair, seq-half, key-tile kt:
       scores_T[k, (h0 q | h1 q)] = k_h @ q_h.T   (K=64 row-packed pair)
       p = exp(scores_T) * exp(mask).T         (mask add via exp-multiply)
       U.T[d,q] += v_h.T @ p   (PSUM accum over kt, heads col-packed M=64)
       sums[q]  += p.T @ ones  (N=1 matmuls, queries on partitions)
  Uhat = U * (1/sums broadcast via PE transpose + K=2 indicator matmul)
  out[q,:] = Uhat.T @ proj_w.T + bias
"""
import functools
import numpy as np
import ml_dtypes
from contextlib import ExitStack

import concourse.bass as bass
import concourse.tile as tile
from concourse import bacc, mybir
from concourse.bass_utils import run_bass_kernel_spmd
from concourse.masks import make_identity

F32 = mybir.dt.float32
F32R = mybir.dt.float32r
BF16 = mybir.dt.bfloat16
AF = mybir.ActivationFunctionType

B, N, DIM, H, D = 4, 2048, 512, 8, 64
SCALE = D ** -0.5
NQ = N // 2          # queries per core
NKT = N // 128       # key tiles (16)
NCORES = 8


def build(dbg=False):
    nc = bacc.Bacc("TRN2", target_bir_lowering=False, debug=False,
                   num_devices=NCORES)
    xT = nc.dram_tensor("xT", [DIM, N], F32R, kind="ExternalInput").ap()
    wqT = nc.dram_tensor("wqT", [DIM, DIM], F32R, kind="ExternalInput").ap()
    wkT = nc.dram_tensor("wkT", [DIM, DIM], F32R, kind="ExternalInput").ap()
    wvT = nc.dram_tensor("wvT", [DIM, DIM], F32R, kind="ExternalInput").ap()
    projT = nc.dram_tensor("projT", [DIM, DIM], F32R, kind="ExternalInput").ap()
    biasb = nc.dram_tensor("biasb", [128, DIM], F32, kind="ExternalInput").ap()
    expmT = nc.dram_tensor("expmT", [N, NQ], BF16, kind="ExternalInput").ap()
    indD = nc.dram_tensor("indD", [2, 128], F32R, kind="ExternalInput").ap()
    out = nc.dram_tensor("out", [NQ, DIM], F32, kind="ExternalOutput").ap()

    with tile.TileContext(nc) as tc, ExitStack() as ctx:
        # ---- SBUF pools ----
        wp = ctx.enter_context(tc.tile_pool(name="wp", bufs=1))
        kv = ctx.enter_context(tc.tile_pool(name="kv", bufs=1))
        small = ctx.enter_context(tc.tile_pool(name="small", bufs=2))
        osb = ctx.enter_context(tc.tile_pool(name="osb", bufs=2))
        # ---- PSUM pools: 4 + 2 + 1 + 1 = 8 banks ----
        ps_stage = ctx.enter_context(
            tc.tile_pool(name="ps_stage", bufs=2, space="PSUM"))   # 2x2 banks
        ps_out = ctx.enter_context(
            tc.tile_pool(name="ps_out", bufs=2, space="PSUM"))     # 2x1 bank
        ps_sums = ctx.enter_context(
            tc.tile_pool(name="ps_sums", bufs=1, space="PSUM"))    # 1 bank
        ps_misc = ctx.enter_context(
            tc.tile_pool(name="ps_misc", bufs=1, space="PSUM"))    # 1 bank

        # ---- constants ----
        ident = wp.tile([128, 128], F32, name="ident", tag="ident")
        make_identity(nc, ident[:])
        ones_bf = wp.tile([128, 1], BF16, name="ones_bf", tag="ones_bf")
        nc.vector.memset(ones_bf[:], 1.0)
        zcol_bf = wp.tile([1, 128], BF16, name="zcol_bf", tag="zcol_bf")
        nc.vector.memset(zcol_bf[:], 0.0)
        zrow_bf = wp.tile([1, 512], BF16, name="zrow_bf", tag="zrow_bf")
        nc.vector.memset(zrow_bf[:], 1.0)
        ind = wp.tile([2, 128], F32R, name="ind", tag="ind")
        nc.sync.dma_start(ind[:], indD[:])

        # ---- persistent loads ----
        pj_sb = [wp.tile([128, DIM], F32R, name=f"pj{kc}", tag=f"pj{kc}")
                 for kc in range(4)]
        for kc in range(4):
            nc.scalar.dma_start(pj_sb[kc][:], projT[kc * 128:(kc + 1) * 128, :])
        bias_sb = wp.tile([128, DIM], F32, name="bias_sb", tag="bias_sb")
        nc.sync.dma_start(bias_sb[:], biasb[:])

        q_sb = [kv.tile([128, NQ], F32R, name=f"q{m}", tag=f"q{m}")
                for m in range(4)]
        k_sb = [kv.tile([128, N], F32R, name=f"k{m}", tag=f"k{m}")
                for m in range(4)]
        v_sb = [kv.tile([128, DIM], BF16, name=f"v{kt}", tag=f"v{kt}")
                for kt in range(NKT)]
        em_sb = [kv.tile([128, NQ], BF16, name=f"em{kt}", tag=f"em{kt}")
                 for kt in range(NKT)]
        for kt in range(NKT):
            nc.scalar.dma_start(em_sb[kt][:], expmT[kt * 128:(kt + 1) * 128, :])
        uhat = [kv.tile([128, NQ], F32R, name=f"uh{p}", tag=f"uh{p}")
                for p in range(4)]

        # ---- phase 1: projections ----
        with tc.tile_pool(name="xp", bufs=1) as xp:
            x_sb = [xp.tile([128, N], F32R, name=f"x{kc}", tag=f"x{kc}")
                    for kc in range(4)]
            wq_sb = [xp.tile([128, DIM], F32R, name=f"wq{kc}", tag=f"wq{kc}")
                     for kc in range(4)]
            wk_sb = [xp.tile([128, DIM], F32R, name=f"wk{kc}", tag=f"wk{kc}")
                     for kc in range(4)]
            wv_sb = [xp.tile([128, DIM], F32R, name=f"wv{kc}", tag=f"wv{kc}")
                     for kc in range(4)]
            for kc in range(4):
                sl = slice(kc * 128, (kc + 1) * 128)
                nc.sync.dma_start(x_sb[kc][:], xT[sl, :])
                nc.scalar.dma_start(wq_sb[kc][:], wqT[sl, :])
                nc.sync.dma_start(wk_sb[kc][:], wkT[sl, :])
                nc.scalar.dma_start(wv_sb[kc][:], wvT[sl, :])

            # q_T [512, 1024] (queries = cols 0:NQ of permuted xT)
            for m in range(4):
                ms = slice(m * 128, (m + 1) * 128)
                ps = ps_stage.tile([128, NQ], F32, name=f"psq{m}", tag="stage")
                for c in range(2):
                    cs = slice(c * 512, (c + 1) * 512)
                    for kc in range(4):
                        nc.tensor.matmul(ps[:, cs], wq_sb[kc][:, ms],
                                         x_sb[kc][:, cs],
                                         start=(kc == 0), stop=(kc == 3))
                nc.vector.tensor_copy(q_sb[m][:], ps[:])

            # k_T [512, 2048]
            for m in range(4):
                ms = slice(m * 128, (m + 1) * 128)
                for half in range(2):
                    ps = ps_stage.tile([128, NQ], F32, name=f"psk{m}_{half}",
                                       tag="stage")
                    for c2 in range(2):
                        cs_o = slice(c2 * 512, (c2 + 1) * 512)
                        cs_x = slice(half * 1024 + c2 * 512,
                                     half * 1024 + (c2 + 1) * 512)
                        for kc in range(4):
                            nc.tensor.matmul(ps[:, cs_o], wk_sb[kc][:, ms],
                                             x_sb[kc][:, cs_x],
                                             start=(kc == 0), stop=(kc == 3))
                    nc.vector.tensor_copy(
                        k_sb[m][:, half * 1024:(half + 1) * 1024], ps[:])

            # v [2048, 512]
            for kt in range(NKT):
                ks = slice(kt * 128, (kt + 1) * 128)
                ps = ps_out.tile([128, DIM], F32, name=f"psv{kt}", tag="out")
                for kc in range(4):
                    nc.tensor.matmul(ps[:], x_sb[kc][:, ks], wv_sb[kc][:],
                                     start=(kc == 0), stop=(kc == 3))
                nc.vector.tensor_copy(v_sb[kt][:], ps[:])

        # ---- phase 2: attention ----
        with tc.tile_pool(name="praw", bufs=3) as praw_p, \
             tc.tile_pool(name="phat", bufs=6) as phat_p:
            for pair in range(4):
                for half in range(2):
                    hq = slice(half * 512, (half + 1) * 512)
                    out_ps = ps_out.tile([128, 512], F32,
                                         name=f"o{pair}_{half}", tag="out")
                    sums_ps = ps_sums.tile([128, 8], F32,
                                           name=f"s{pair}_{half}", tag="sums")
                    # open ONE accumulation group over the sums bank
                    nc.tensor.matmul(sums_ps[:], zcol_bf[:], zrow_bf[:, 0:8],
                                     start=True, stop=False)
                    for kt in range(NKT):
                        kts = slice(kt * 128, (kt + 1) * 128)
                        st = ps_stage.tile([128, 1024], F32,
                                           name=f"st{pair}_{half}_{kt}",
                                           tag="stage")
                        # scores: both heads adjacent -> row-packed pair
                        for hi in range(2):
                            po = hi * 64
                            pos = slice(po, po + 64)
                            nc.tensor.matmul(
                                st[:, hi * 512:(hi + 1) * 512],
                                k_sb[pair][pos, kts], q_sb[pair][pos, hq],
                                start=True, stop=True, tile_position=(po, 0))
                        praw = praw_p.tile([128, 1024], BF16,
                                           name=f"pr{pair}_{half}_{kt}",
                                           tag="pr")
                        nc.scalar.activation(praw[:], st[:], AF.Exp)
                        phat = phat_p.tile([128, 1024], BF16,
                                           name=f"ph{pair}_{half}_{kt}",
                                           tag="ph")
                        em2 = em_sb[kt][:, hq].rearrange(
                            "p (o f) -> p o f", o=1).broadcast_to([128, 2, 512])
                        nc.vector.tensor_mul(
                            phat[:].rearrange("p (t f) -> p t f", t=2),
                            praw[:].rearrange("p (t f) -> p t f", t=2), em2)
                        # p@v: col-packed pair
                        for hi in range(2):
                            h = 2 * pair + hi
                            po = hi * 64
                            pos = slice(po, po + 64)
                            nc.tensor.matmul(
                                out_ps[pos, :],
                                v_sb[kt][:, h * 64:(h + 1) * 64],
                                phat[:, hi * 512:(hi + 1) * 512],
                                start=(kt == 0), stop=(kt == NKT - 1),
                                skip_group_check=(hi == 1),
                                tile_position=(0, po))
                        # sums: 8 N=1 matmuls
                        for col in range(8):
                            nc.tensor.matmul(
                                sums_ps[:, col:col + 1],
                                phat[:, col * 128:(col + 1) * 128],
                                ones_bf[:],
                                start=False,
                                stop=(kt == NKT - 1 and col == 7),
                                skip_group_check=True)

                    # normalize: r = 1/sums broadcast along the free dim
                    sums_sb = small.tile([128, 8], F32,
                                         name=f"ss{pair}_{half}", tag="ss")
                    nc.vector.tensor_copy(sums_sb[:], sums_ps[:])
                    rrec = small.tile([128, 8], F32,
                                      name=f"rr{pair}_{half}", tag="rr")
                    nc.vector.reciprocal(rrec[:], sums_sb[:])
                    rT_ps = ps_misc.tile([8, 128], F32,
                                         name=f"rt{pair}_{half}", tag="misc")
                    nc.tensor.transpose(rT_ps[:], rrec[:], ident[:])
                    rT_sb = small.tile([8, 128], F32R,
                                       name=f"rs{pair}_{half}", tag="rs")
                    nc.vector.tensor_copy(rT_sb[:], rT_ps[:])
                    rrow = small.tile([2, 4 * 128], F32R,
                                      name=f"rw{pair}_{half}", tag="rw")
                    for j in range(8):
                        hi, m = j // 4, j % 4
                        nc.sync.dma_start(
                            rrow[hi:hi + 1, m * 128:(m + 1) * 128],
                            rT_sb[j:j + 1, :])
                    bc_ps = ps_misc.tile([128, 512], F32,
                                         name=f"bc{pair}_{half}", tag="misc")
                    nc.tensor.matmul(bc_ps[:], ind[:], rrow[0:2, :],
                                     start=True, stop=True)
                    bc_sb = small.tile([128, 512], F32,
                                       name=f"bs{pair}_{half}", tag="bs")
                    nc.vector.tensor_copy(bc_sb[:], bc_ps[:])
                    nc.vector.tensor_mul(uhat[pair][:, hq], out_ps[:],
                                         bc_sb[:])

        # ---- phase 3: output projection ----
        for m in range(8):
            ms = slice(m * 128, (m + 1) * 128)
            pp = ps_stage.tile([128, DIM], F32, name=f"pp{m}", tag="stage")
            for kc in range(4):
                nc.tensor.matmul(pp[:], uhat[kc][:, ms], pj_sb[kc][:],
                                 start=(kc == 0), stop=(kc == 3))
            ob = osb.tile([128, DIM], F32, name=f"ob{m}", tag="ob")
            nc.vector.tensor_add(ob[:], pp[:], bias_sb[:])
            nc.sync.dma_start(out[ms, :], ob[:])

    nc.compile()
    return nc


@functools.lru_cache(maxsize=1)
def _get_nc():
    return build()


def _prep_inputs(x, attn_mask, qkv_w, proj_w, proj_b):
    x = np.asarray(x, dtype=np.float32)
    mask = np.asarray(attn_mask, dtype=np.float32).reshape(N, N)
    qkv_w = np.asarray(qkv_w, dtype=np.float32)
    proj_w = np.asarray(proj_w, dtype=np.float32)
    proj_b = np.asarray(proj_b, dtype=np.float32)

    wqT = np.ascontiguousarray((qkv_w[0:DIM] * SCALE).T)
    wkT = np.ascontiguousarray(qkv_w[DIM:2 * DIM].T)
    wvT = np.ascontiguousarray(qkv_w[2 * DIM:3 * DIM].T)
    projT = np.ascontiguousarray(proj_w.T)
    biasb = np.tile(proj_b, (128, 1))
    indmat = np.zeros((2, 128), dtype=np.float32)
    indmat[0, 0:64] = 1.0
    indmat[1, 64:128] = 1.0

    expm = np.exp(mask)
    # per-core key permutation: own seq-half first, other half second, so
    # the query chunk is always columns [0, NQ) of the permuted x.T
    xTs = {}
    emTs = {}
    for s in range(2):
        o = 1 - s
        emT = np.ascontiguousarray(expm[s * NQ:(s + 1) * NQ, :].T)  # [keys, q]
        emTs[s] = np.concatenate(
            [emT[s * NQ:(s + 1) * NQ], emT[o * NQ:(o + 1) * NQ]], axis=0
        ).astype(ml_dtypes.bfloat16)
        for b in range(B):
            xTb = x[b].T  # [DIM, N]
            xTs[(b, s)] = np.ascontiguousarray(np.concatenate(
                [xTb[:, s * NQ:(s + 1) * NQ], xTb[:, o * NQ:(o + 1) * NQ]],
                axis=1))

    in_maps = []
    for c in range(NCORES):
        b, s = c // 2, c % 2
        in_maps.append({
            "xT": xTs[(b, s)],
            "wqT": wqT, "wkT": wkT, "wvT": wvT, "projT": projT,
            "biasb": biasb, "expmT": emTs[s], "indD": indmat,
        })
    return in_maps


def run(inputs, trace=False, tmpdir=None):
    nc = _get_nc()
    in_maps = _prep_inputs(**inputs)
    res = run_bass_kernel_spmd(nc, in_maps, core_ids=list(range(NCORES)),
                               trace=trace, tmpdir=tmpdir)
    full = np.empty((B, N, DIM), dtype=np.float32)
    for c in range(NCORES):
        b, s = c // 2, c % 2
        full[b, s * NQ:(s + 1) * NQ, :] = res.results[c]["out"]
    return full, res


def kernel(**inputs) -> np.ndarray:
    return run(inputs)[0]
